# revision 11
# baseline (speedup 1.0000x reference)
"""HeatmapMSELoss Trainium2 kernel (mixed fp8/bf16, 3-engine version).

Computes mean((heatmaps_pred - heatmaps_gt)^2) where heatmaps_gt is an
isotropic 2D gaussian (sigma=1, peak 1) rendered at the projection of each
3D joint into each view.

Key identity: the gaussian separates, gt[h,w] = gy[h] * gx[w], so

  sum_hw (pred - gt)^2 = sum_hw pred^2 - 2 * gy^T (pred @ gx) + (sum gy^2)(sum gx^2)

The gt tensor is never materialized. pred is pre-transposed on host to
h-major [H, S, W] (so every DMA descriptor is >= 512 contiguous bytes)
and split into two streams: an fp8e4m3 stream whose squares are summed by
the ACT engine (activation Square + accumulator) and the Pool engine
(gpsimd multiply + halving adds into a persistent f32 accumulator), and a
bf16 stream whose squares are summed by the DVE engine (tensor_mul at 2x
bf16 rate + halving tree + short reduce). This keeps all three
element-wise engines busy in parallel, which is what bounds the kernel
(the quantized streams need only ~17us of DMA).

fp8 quantization biases sum(pred^2) by ~+5e-4 relative (mean squared
rounding error); harness tolerance is 2e-2.

The cross term uses one per-slice matmul (pred_s^T @ gy_s) into a
persistent PSUM tile; the multiply by gx and reduction run as two grouped
DVE ops over hundreds of columns at once.

Sharding: data-parallel over batch, 4 batches per core across 8 cores;
per-chunk column partials are combined on host in float64.
"""

import numpy as np
import ml_dtypes

import concourse.bacc as bacc
import concourse.bass as bass
import concourse.tile as tile
from concourse import mybir
from concourse.bass_utils import run_bass_kernel_spmd

B, V, J, H, W = 32, 4, 17, 128, 128
N_CORES = 8
B_LOC = B // N_CORES          # 4 batches per core
SLICES = B_LOC * V * J        # 272 slices per core

_CACHE = {}

# Per round: (act_slices, dve_slices, pool_slices).
# ACT+Pool slices stream as fp8, DVE slices as bf16. Round sizes decrease
# toward the end so the last round's compute is short after its DMA lands.
ROUNDS = [
    (10, 8, 4),
    (27, 18, 4),
    (27, 18, 4),
    (26, 17, 4),
    (24, 16, 4),
    (24, 17, 4),
    (8, 6, 2),
]
assert sum(a + d + p for a, d, p in ROUNDS) == SLICES
NR = len(ROUNDS)
MAXA = max(a for a, _, _ in ROUNDS)
MAXD = max(d for _, d, _ in ROUNDS)
MAXP = max(p for _, _, p in ROUNDS)
S8 = sum(a + p for a, _, p in ROUNDS)   # fp8 slices total
S16 = sum(d for _, d, _ in ROUNDS)      # bf16 slices total

# outcols layout: [ACT accums (NR) | DVE reduces (NR) | poolacc reduce (1) |
#                  prodA (1) | prodB (1)]
NC = 2 * NR + 3
# psA holds m' columns for rounds 0..NGRP-1, psB the rest; prodA is issued
# inside round NGRP's iteration (its matmul wait resolves before round
# NGRP's own DVE work, so it never head-of-line blocks the DVE queue)
NGRP = NR - 2


def _build_nc():
    nc = bacc.Bacc()
    f32 = mybir.dt.float32
    bf16 = mybir.dt.bfloat16
    fp8 = mybir.dt.float8e4

    pred8 = nc.declare_dram_parameter("pred8", [H, S8, W], fp8, isOutput=False)
    pred16 = nc.declare_dram_parameter("pred16", [H, S16, W], bf16, isOutput=False)
    gy8 = nc.declare_dram_parameter("gy8", [H, S8], fp8, isOutput=False)
    gy16 = nc.declare_dram_parameter("gy16", [H, S16], bf16, isOutput=False)
    gx = nc.declare_dram_parameter("gx", [W, SLICES], bf16, isOutput=False)
    partials = nc.declare_dram_parameter("partials", [128, NC], f32, isOutput=True)

    # columns in the persistent PSUM m' tiles follow global slice order:
    # per round, fp8 slices (ACT then Pool shares) first, then bf16 slices.
    n_main = sum(sum(r) for r in ROUNDS[:NGRP])
    n_last = SLICES - n_main

    with tile.TileContext(nc) as tc:
        with (
            tc.tile_pool(name="consts", bufs=1) as consts,
            tc.tile_pool(name="l8", bufs=5) as l8pool,
            tc.tile_pool(name="l16", bufs=5) as l16pool,
            tc.tile_pool(name="work", bufs=1) as work,
            tc.tile_pool(name="psum", bufs=1, space="PSUM") as psumpool,
            tc.tile_pool(name="outs", bufs=1) as outs,
        ):
            # warm-up ACT so the Square table-set load overlaps the first DMA
            warm = consts.tile([128, 1], f32)
            nc.vector.memset(warm[:], 0.0)
            wsq = consts.tile([128, 1], f32)
            nc.scalar.activation(
                out=wsq[:], in_=warm[:], func=mybir.ActivationFunctionType.Square
            )

            gy8_t = consts.tile([H, S8], fp8)
            gy16_t = consts.tile([H, S16], bf16)
            gx_t = consts.tile([W, SLICES], bf16)
            actout = consts.tile([128, MAXA * W], bf16)
            sq16 = consts.tile([128, MAXD * W], bf16)
            sq8 = consts.tile([128, MAXP * W], bf16)
            poolacc = consts.tile([128, 256], f32)
            nc.gpsimd.memset(poolacc[:], 0.0)
            outcols = outs.tile([128, NC], f32)

            psA = psumpool.tile([128, n_main], f32, tag="psA")
            psB = psumpool.tile([128, n_last], f32, tag="psB")

            s8_0 = 0   # running fp8 slice offset
            s16_0 = 0  # running bf16 slice offset
            g0 = 0     # running global slice offset (psum/gx column order)
            for r, (ak, dk, pk) in enumerate(ROUNDS):
                fk = ak + pk  # fp8 slices this round
                # t8 first (ACT is the busiest engine, start it earliest);
                # last round loads t16 first since DVE's tail chain is longer
                t16 = l16pool.tile([128, MAXD * W], bf16, tag="l16")
                t8 = l8pool.tile([128, (MAXA + MAXP) * W], fp8, tag="l8")

                def load16():
                    nc.sync.dma_start(
                        out=t16[:, : dk * W],
                        in_=pred16[:, s16_0 : s16_0 + dk, :].rearrange(
                            "h s w -> h (s w)"
                        ),
                    )

                def load8():
                    nc.sync.dma_start(
                        out=t8[:, : fk * W],
                        in_=pred8[:, s8_0 : s8_0 + fk, :].rearrange(
                            "h s w -> h (s w)"
                        ),
                    )

                if r == NR - 1:
                    load16(); load8()
                else:
                    load8(); load16()
                if r == 0:
                    # small gaussian loads slot in after the first round's
                    # pred DMAs so the main stream starts immediately
                    nc.sync.dma_start(out=gy8_t[:], in_=gy8[:, :])
                    nc.sync.dma_start(out=gy16_t[:], in_=gy16[:, :])
                    nc.sync.dma_start(out=gx_t[:], in_=gx[:, :])

                # ACT: sum of squares over its fp8 share -> outcols[r]
                nc.scalar.activation(
                    out=actout[:, : ak * W],
                    in_=t8[:, : ak * W],
                    func=mybir.ActivationFunctionType.Square,
                    accum_out=outcols[:, r : r + 1],
                )

                # DVE: square bf16 share at 2x, halving tree, short reduce
                n = dk * W
                nc.vector.tensor_mul(sq16[:, :n], t16[:, :n], t16[:, :n])
                while n > 256:
                    h = n // 2
                    nc.vector.tensor_add(sq16[:, :h], sq16[:, :h], sq16[:, h:n])
                    n = h
                nc.vector.reduce_sum(
                    outcols[:, NR + r : NR + r + 1], sq16[:, :n],
                    axis=mybir.AxisListType.X,
                )

                # Pool: square fp8 share, halve to 256, add into poolacc
                n = pk * W
                nc.gpsimd.tensor_mul(
                    sq8[:, :n], t8[:, ak * W : fk * W], t8[:, ak * W : fk * W]
                )
                while n > 256:
                    h = n // 2
                    nc.gpsimd.tensor_add(sq8[:, :h], sq8[:, :h], sq8[:, h:n])
                    n = h
                nc.gpsimd.tensor_add(
                    poolacc[:, :n], poolacc[:, :n], sq8[:, :n]
                )

                # cross term: m'_s = pred_s^T @ gy_s per slice -> psum column
                ps, col0 = (psA, g0) if r < NGRP else (psB, g0 - n_main)
                for i in range(fk):
                    nc.tensor.matmul(
                        ps[:, col0 + i : col0 + i + 1],
                        t8[:, i * W : (i + 1) * W],
                        gy8_t[:, s8_0 + i : s8_0 + i + 1],
                        start=True,
                        stop=True,
                    )
                for j in range(dk):
                    nc.tensor.matmul(
                        ps[:, col0 + fk + j : col0 + fk + j + 1],
                        t16[:, j * W : (j + 1) * W],
                        gy16_t[:, s16_0 + j : s16_0 + j + 1],
                        start=True,
                        stop=True,
                    )

                if r == NGRP:
                    # grouped prod over the main rounds' m' columns; its
                    # matmul wait resolved before this round's DVE work, so
                    # it slots into the DVE queue without blocking it
                    prodA = work.tile([128, n_main], f32, tag="prodA")
                    nc.vector.tensor_mul(prodA[:], psA[:], gx_t[:, :n_main])
                    nc.vector.reduce_sum(
                        outcols[:, 2 * NR + 1 : 2 * NR + 2], prodA[:],
                        axis=mybir.AxisListType.X,
                    )

                s8_0 += fk
                s16_0 += dk
                g0 += ak + dk + pk

            # tail: pool accumulator reduce, last-round prod
            nc.vector.reduce_sum(
                outcols[:, 2 * NR : 2 * NR + 1], poolacc[:],
                axis=mybir.AxisListType.X,
            )
            prodB = work.tile([128, n_last], f32, tag="prodB")
            nc.vector.tensor_mul(prodB[:], psB[:], gx_t[:, n_main:])
            nc.vector.reduce_sum(
                outcols[:, 2 * NR + 2 : 2 * NR + 3], prodB[:],
                axis=mybir.AxisListType.X,
            )

            nc.sync.dma_start(out=partials[:, :], in_=outcols[:])

    nc.finalize()
    return nc


def _gaussians(proj_mats_batch, joints_3d_gt_batch):
    """1D gaussians gy [B,V,J,H], gx [B,V,J,W] in float32 (reference math)."""
    joints = joints_3d_gt_batch.astype(np.float32)
    ones = np.ones(joints.shape[:-1] + (1,), dtype=np.float32)
    joints_h = np.concatenate([joints, ones], axis=-1)  # [B, J, 4]
    proj = np.einsum(
        "bvcd,bjd->bvjc", proj_mats_batch.astype(np.float32), joints_h
    ).astype(np.float32)  # [B, V, J, 3]
    joints_2d = proj[..., :2] / proj[..., 2:3]  # (x, y)
    xs = np.arange(W, dtype=np.float32)
    ys = np.arange(H, dtype=np.float32)
    dx2 = (xs - joints_2d[..., 0, None]) ** 2  # [B,V,J,W]
    dy2 = (ys - joints_2d[..., 1, None]) ** 2  # [B,V,J,H]
    gx = np.exp(-0.5 * dx2).astype(np.float32)
    gy = np.exp(-0.5 * dy2).astype(np.float32)
    return gy, gx


def _split_masks():
    """Boolean masks over the 272 per-core slices: fp8 vs bf16 stream."""
    m8 = np.zeros(SLICES, dtype=bool)
    g0 = 0
    for ak, dk, pk in ROUNDS:
        m8[g0 : g0 + ak + pk] = True       # fp8: ACT share then Pool share
        g0 += ak + pk + dk                 # bf16 share follows
    return m8, ~m8


def kernel(heatmaps_pred, proj_mats_batch, joints_3d_gt_batch, joints_3d_valid_batch,
           _profile=None):
    heatmaps_pred = np.asarray(heatmaps_pred, dtype=np.float32)
    gy, gx = _gaussians(np.asarray(proj_mats_batch), np.asarray(joints_3d_gt_batch))

    # s3 = sum over slices of (sum_h gy^2) * (sum_w gx^2), exact in f64
    s3 = float(
        ((gy.astype(np.float64) ** 2).sum(-1) * (gx.astype(np.float64) ** 2).sum(-1)).sum()
    )

    if "nc" not in _CACHE:
        _CACHE["nc"] = _build_nc()
    nc = _CACHE["nc"]

    m8, m16 = _split_masks()
    in_maps = []
    for c in range(N_CORES):
        bsl = slice(B_LOC * c, B_LOC * (c + 1))
        # slice order: (b_local, v, j) -> s ; pred h-major [H, n, W]
        pred_c = heatmaps_pred[bsl].reshape(SLICES, H, W)
        pred8 = np.ascontiguousarray(
            pred_c[m8].transpose(1, 0, 2).astype(ml_dtypes.float8_e4m3)
        )
        pred16 = np.ascontiguousarray(
            pred_c[m16].transpose(1, 0, 2).astype(ml_dtypes.bfloat16)
        )
        gy_c = gy[bsl].reshape(SLICES, H)
        gy8 = np.ascontiguousarray(gy_c[m8].T.astype(ml_dtypes.float8_e4m3))
        gy16 = np.ascontiguousarray(gy_c[m16].T.astype(ml_dtypes.bfloat16))
        gx_c = np.ascontiguousarray(
            gx[bsl].reshape(SLICES, W).T.astype(ml_dtypes.bfloat16)
        )
        in_maps.append(
            {"pred8": pred8, "pred16": pred16, "gy8": gy8, "gy16": gy16,
             "gx": gx_c}
        )

    res = run_bass_kernel_spmd(nc, in_maps, core_ids=list(range(N_CORES)))
    if _profile is not None:
        _profile["result"] = res
        _profile["in_maps"] = in_maps

    s1 = 0.0
    s2 = 0.0
    for c in range(N_CORES):
        p = res.results[c]["partials"].astype(np.float64)
        s1 += p[:, : 2 * NR + 1].sum()
        s2 += p[:, 2 * NR + 1 :].sum()

    total = s1 - 2.0 * s2 + s3
    return np.float32(total / (B * V * J * H * W))


# revision 14
# speedup vs baseline: 1.0331x; 1.0331x over previous
"""HeatmapMSELoss Trainium2 kernel (mixed fp8/bf16, 3-engine version).

Computes mean((heatmaps_pred - heatmaps_gt)^2) where heatmaps_gt is an
isotropic 2D gaussian (sigma=1, peak 1) rendered at the projection of each
3D joint into each view.

Key identity: the gaussian separates, gt[h,w] = gy[h] * gx[w], so

  sum_hw (pred - gt)^2 = sum_hw pred^2 - 2 * gy^T (pred @ gx) + (sum gy^2)(sum gx^2)

The gt tensor is never materialized. pred is pre-transposed on host to
h-major [H, S, W] (so every DMA descriptor is >= 512 contiguous bytes)
and split into two streams: an fp8e4m3 stream whose squares are summed by
the ACT engine (activation Square + accumulator) and the Pool engine
(gpsimd multiply + halving adds into a persistent f32 accumulator), and a
bf16 stream whose squares are summed by the DVE engine (tensor_mul at 2x
bf16 rate + halving tree + short reduce). This keeps all three
element-wise engines busy in parallel, which is what bounds the kernel
(the quantized streams need only ~17us of DMA).

fp8 quantization biases sum(pred^2) by ~+5e-4 relative (mean squared
rounding error); harness tolerance is 2e-2.

The cross term uses one per-slice matmul (pred_s^T @ gy_s) into a
persistent PSUM tile; the multiply by gx and reduction run as two grouped
DVE ops over hundreds of columns at once.

Sharding: data-parallel over batch, 4 batches per core across 8 cores;
per-chunk column partials are combined on host in float64.
"""

import numpy as np
import ml_dtypes

import concourse.bacc as bacc
import concourse.bass as bass
import concourse.tile as tile
from concourse import mybir
from concourse.bass_utils import run_bass_kernel_spmd

B, V, J, H, W = 32, 4, 17, 128, 128
N_CORES = 8
B_LOC = B // N_CORES          # 4 batches per core
SLICES = B_LOC * V * J        # 272 slices per core

_CACHE = {}

# Per round: (act_slices, dve_slices, pool_slices).
# ACT+Pool slices stream as fp8, DVE slices as bf16. Round sizes decrease
# toward the end so the last round's compute is short after its DMA lands.
ROUNDS = [
    (12, 10, 4),
    (24, 18, 4),
    (24, 18, 4),
    (24, 17, 4),
    (23, 17, 4),
    (23, 17, 4),
    (10, 9, 2),
]
assert sum(a + d + p for a, d, p in ROUNDS) == SLICES
NR = len(ROUNDS)
MAXA = max(a for a, _, _ in ROUNDS)
MAXD = max(d for _, d, _ in ROUNDS)
MAXP = max(p for _, _, p in ROUNDS)
S8 = sum(a + p for a, _, p in ROUNDS)   # fp8 slices total
S16 = sum(d for _, d, _ in ROUNDS)      # bf16 slices total

# outcols layout: [ACT accums (NR) | DVE reduces (NR) | poolacc reduce (1) |
#                  prodA (1) | prodB (1)]
NC = 2 * NR + 3
# psA holds m' columns for rounds 0..NGRP-1, psB the rest; prodA is issued
# inside round NGRP's iteration (its matmul wait resolves before round
# NGRP's own DVE work, so it never head-of-line blocks the DVE queue)
NGRP = NR - 2


def _build_nc():
    nc = bacc.Bacc()
    f32 = mybir.dt.float32
    bf16 = mybir.dt.bfloat16
    fp8 = mybir.dt.float8e4

    pred8 = nc.declare_dram_parameter("pred8", [H, S8, W], fp8, isOutput=False)
    pred16 = nc.declare_dram_parameter("pred16", [H, S16, W], bf16, isOutput=False)
    gy8 = nc.declare_dram_parameter("gy8", [H, S8], fp8, isOutput=False)
    gy16 = nc.declare_dram_parameter("gy16", [H, S16], bf16, isOutput=False)
    gx = nc.declare_dram_parameter("gx", [W, SLICES], bf16, isOutput=False)
    partials = nc.declare_dram_parameter("partials", [128, NC], f32, isOutput=True)

    # columns in the persistent PSUM m' tiles follow global slice order:
    # per round, fp8 slices (ACT then Pool shares) first, then bf16 slices.
    n_main = sum(sum(r) for r in ROUNDS[:NGRP])
    n_last = SLICES - n_main

    with tile.TileContext(nc) as tc:
        with (
            tc.tile_pool(name="consts", bufs=1) as consts,
            tc.tile_pool(name="l8", bufs=5) as l8pool,
            tc.tile_pool(name="l16", bufs=5) as l16pool,
            tc.tile_pool(name="work", bufs=1) as work,
            tc.tile_pool(name="psum", bufs=1, space="PSUM") as psumpool,
            tc.tile_pool(name="outs", bufs=1) as outs,
        ):
            # warm-up ACT so the Square table-set load overlaps the first DMA
            warm = consts.tile([128, 1], f32)
            nc.vector.memset(warm[:], 0.0)
            wsq = consts.tile([128, 1], f32)
            nc.scalar.activation(
                out=wsq[:], in_=warm[:], func=mybir.ActivationFunctionType.Square
            )

            gy8_t = consts.tile([H, S8], fp8)
            gy16_t = consts.tile([H, S16], bf16)
            gx_t = consts.tile([W, SLICES], bf16)
            actout = consts.tile([128, MAXA * W], bf16)
            sq16 = consts.tile([128, MAXD * W], bf16)
            sq8 = consts.tile([128, MAXP * W], bf16)
            poolacc = consts.tile([128, 256], f32)
            nc.gpsimd.memset(poolacc[:], 0.0)
            outcols = outs.tile([128, NC], f32)

            psA = psumpool.tile([128, n_main], f32, tag="psA")
            psB = psumpool.tile([128, n_last], f32, tag="psB")

            s8_0 = 0   # running fp8 slice offset
            s16_0 = 0  # running bf16 slice offset
            g0 = 0     # running global slice offset (psum/gx column order)
            for r, (ak, dk, pk) in enumerate(ROUNDS):
                fk = ak + pk  # fp8 slices this round
                # t8 first (ACT is the busiest engine, start it earliest);
                # last round loads t16 first since DVE's tail chain is longer
                t16 = l16pool.tile([128, MAXD * W], bf16, tag="l16")
                t8 = l8pool.tile([128, (MAXA + MAXP) * W], fp8, tag="l8")

                def load16():
                    nc.sync.dma_start(
                        out=t16[:, : dk * W],
                        in_=pred16[:, s16_0 : s16_0 + dk, :].rearrange(
                            "h s w -> h (s w)"
                        ),
                    )

                def load8():
                    nc.sync.dma_start(
                        out=t8[:, : fk * W],
                        in_=pred8[:, s8_0 : s8_0 + fk, :].rearrange(
                            "h s w -> h (s w)"
                        ),
                    )

                if r == NR - 1:
                    load16(); load8()
                else:
                    load8(); load16()
                if r == 1:
                    # small gaussian loads slot in after the first two
                    # rounds' pred DMAs; only the (slack-rich) PE matmuls
                    # and the late grouped prods consume them
                    nc.sync.dma_start(out=gy8_t[:], in_=gy8[:, :])
                    nc.sync.dma_start(out=gy16_t[:], in_=gy16[:, :])
                    nc.sync.dma_start(out=gx_t[:], in_=gx[:, :])

                # ACT: sum of squares over its fp8 share -> outcols[r]
                nc.scalar.activation(
                    out=actout[:, : ak * W],
                    in_=t8[:, : ak * W],
                    func=mybir.ActivationFunctionType.Square,
                    accum_out=outcols[:, r : r + 1],
                )

                # DVE: square bf16 share at 2x, halving tree, short reduce
                n = dk * W
                nc.vector.tensor_mul(sq16[:, :n], t16[:, :n], t16[:, :n])
                while n > 256:
                    h = n // 2
                    nc.vector.tensor_add(sq16[:, :h], sq16[:, :h], sq16[:, h:n])
                    n = h
                nc.vector.reduce_sum(
                    outcols[:, NR + r : NR + r + 1], sq16[:, :n],
                    axis=mybir.AxisListType.X,
                )

                # Pool: square fp8 share, halve to 256, add into poolacc
                n = pk * W
                nc.gpsimd.tensor_mul(
                    sq8[:, :n], t8[:, ak * W : fk * W], t8[:, ak * W : fk * W]
                )
                while n > 256:
                    h = n // 2
                    nc.gpsimd.tensor_add(sq8[:, :h], sq8[:, :h], sq8[:, h:n])
                    n = h
                nc.gpsimd.tensor_add(
                    poolacc[:, :n], poolacc[:, :n], sq8[:, :n]
                )

                # cross term: m'_s = pred_s^T @ gy_s per slice -> psum column
                ps, col0 = (psA, g0) if r < NGRP else (psB, g0 - n_main)

                def emit_matmuls(ps=ps, col0=col0, t8=t8, t16=t16, fk=fk,
                                 dk=dk, s8_0=s8_0, s16_0=s16_0):
                    for i in range(fk):
                        nc.tensor.matmul(
                            ps[:, col0 + i : col0 + i + 1],
                            t8[:, i * W : (i + 1) * W],
                            gy8_t[:, s8_0 + i : s8_0 + i + 1],
                            start=True,
                            stop=True,
                        )
                    for j in range(dk):
                        nc.tensor.matmul(
                            ps[:, col0 + fk + j : col0 + fk + j + 1],
                            t16[:, j * W : (j + 1) * W],
                            gy16_t[:, s16_0 + j : s16_0 + j + 1],
                            start=True,
                            stop=True,
                        )

                if r == 0:
                    # gy loads are only issued during round 1; defer round
                    # 0's matmuls so they come after in program order
                    mm0 = emit_matmuls
                else:
                    if r == 1:
                        mm0()
                    emit_matmuls()

                if r == NGRP:
                    # grouped prod over the main rounds' m' columns; its
                    # matmul wait resolved before this round's DVE work, so
                    # it slots into the DVE queue without blocking it
                    prodA = work.tile([128, n_main], f32, tag="prodA")
                    nc.vector.tensor_mul(prodA[:], psA[:], gx_t[:, :n_main])
                    nc.vector.reduce_sum(
                        outcols[:, 2 * NR + 1 : 2 * NR + 2], prodA[:],
                        axis=mybir.AxisListType.X,
                    )

                s8_0 += fk
                s16_0 += dk
                g0 += ak + dk + pk

            # tail: pool accumulator reduce, last-round prod
            nc.vector.reduce_sum(
                outcols[:, 2 * NR : 2 * NR + 1], poolacc[:],
                axis=mybir.AxisListType.X,
            )
            prodB = work.tile([128, n_last], f32, tag="prodB")
            nc.vector.tensor_mul(prodB[:], psB[:], gx_t[:, n_main:])
            nc.vector.reduce_sum(
                outcols[:, 2 * NR + 2 : 2 * NR + 3], prodB[:],
                axis=mybir.AxisListType.X,
            )

            nc.sync.dma_start(out=partials[:, :], in_=outcols[:])

    nc.finalize()
    return nc


def _gaussians(proj_mats_batch, joints_3d_gt_batch):
    """1D gaussians gy [B,V,J,H], gx [B,V,J,W] in float32 (reference math)."""
    joints = joints_3d_gt_batch.astype(np.float32)
    ones = np.ones(joints.shape[:-1] + (1,), dtype=np.float32)
    joints_h = np.concatenate([joints, ones], axis=-1)  # [B, J, 4]
    proj = np.einsum(
        "bvcd,bjd->bvjc", proj_mats_batch.astype(np.float32), joints_h
    ).astype(np.float32)  # [B, V, J, 3]
    joints_2d = proj[..., :2] / proj[..., 2:3]  # (x, y)
    xs = np.arange(W, dtype=np.float32)
    ys = np.arange(H, dtype=np.float32)
    dx2 = (xs - joints_2d[..., 0, None]) ** 2  # [B,V,J,W]
    dy2 = (ys - joints_2d[..., 1, None]) ** 2  # [B,V,J,H]
    gx = np.exp(-0.5 * dx2).astype(np.float32)
    gy = np.exp(-0.5 * dy2).astype(np.float32)
    return gy, gx


def _split_masks():
    """Boolean masks over the 272 per-core slices: fp8 vs bf16 stream."""
    m8 = np.zeros(SLICES, dtype=bool)
    g0 = 0
    for ak, dk, pk in ROUNDS:
        m8[g0 : g0 + ak + pk] = True       # fp8: ACT share then Pool share
        g0 += ak + pk + dk                 # bf16 share follows
    return m8, ~m8


def kernel(heatmaps_pred, proj_mats_batch, joints_3d_gt_batch, joints_3d_valid_batch,
           _profile=None):
    heatmaps_pred = np.asarray(heatmaps_pred, dtype=np.float32)
    gy, gx = _gaussians(np.asarray(proj_mats_batch), np.asarray(joints_3d_gt_batch))

    # s3 = sum over slices of (sum_h gy^2) * (sum_w gx^2), exact in f64
    s3 = float(
        ((gy.astype(np.float64) ** 2).sum(-1) * (gx.astype(np.float64) ** 2).sum(-1)).sum()
    )

    if "nc" not in _CACHE:
        _CACHE["nc"] = _build_nc()
    nc = _CACHE["nc"]

    m8, m16 = _split_masks()
    in_maps = []
    for c in range(N_CORES):
        bsl = slice(B_LOC * c, B_LOC * (c + 1))
        # slice order: (b_local, v, j) -> s ; pred h-major [H, n, W]
        pred_c = heatmaps_pred[bsl].reshape(SLICES, H, W)
        pred8 = np.ascontiguousarray(
            pred_c[m8].transpose(1, 0, 2).astype(ml_dtypes.float8_e4m3)
        )
        pred16 = np.ascontiguousarray(
            pred_c[m16].transpose(1, 0, 2).astype(ml_dtypes.bfloat16)
        )
        gy_c = gy[bsl].reshape(SLICES, H)
        gy8 = np.ascontiguousarray(gy_c[m8].T.astype(ml_dtypes.float8_e4m3))
        gy16 = np.ascontiguousarray(gy_c[m16].T.astype(ml_dtypes.bfloat16))
        gx_c = np.ascontiguousarray(
            gx[bsl].reshape(SLICES, W).T.astype(ml_dtypes.bfloat16)
        )
        in_maps.append(
            {"pred8": pred8, "pred16": pred16, "gy8": gy8, "gy16": gy16,
             "gx": gx_c}
        )

    res = run_bass_kernel_spmd(nc, in_maps, core_ids=list(range(N_CORES)))
    if _profile is not None:
        _profile["result"] = res
        _profile["in_maps"] = in_maps

    s1 = 0.0
    s2 = 0.0
    for c in range(N_CORES):
        p = res.results[c]["partials"].astype(np.float64)
        s1 += p[:, : 2 * NR + 1].sum()
        s2 += p[:, 2 * NR + 1 :].sum()

    total = s1 - 2.0 * s2 + s3
    return np.float32(total / (B * V * J * H * W))


# revision 16
# speedup vs baseline: 1.2777x; 1.2368x over previous
"""HeatmapMSELoss Trainium2 kernel (fp8 stream + PE Gram-matmul squares).

Computes mean((heatmaps_pred - heatmaps_gt)^2) where heatmaps_gt is an
isotropic 2D gaussian (sigma=1, peak 1) rendered at the projection of each
3D joint into each view.

Key identity: the gaussian separates, gt[h,w] = gy[h] * gx[w], so

  sum_hw (pred - gt)^2 = sum_hw pred^2 - 2 * gy^T (pred @ gx) + (sum gy^2)(sum gx^2)

The gt tensor is never materialized. pred is pre-transposed on host to
h-major [H, S, W] and cast to fp8e4m3 (biases the final scalar by ~7e-4
relative, harness tolerance 2e-2), quartering HBM traffic vs f32 while
every DMA descriptor stays >= 512 contiguous bytes.

sum(pred^2) rides the (otherwise idle) PE systolic array: per group of
slices, PSUM-accumulated Gram matmuls G = sum_s pred_s^T pred_s, whose
trace is sum of squares. The diagonal is extracted once per group by one
DVE multiply against a preloaded identity mask and one reduce. The ACT
engine squares a minority share directly (activation Square + accum),
sized larger in the early groups while the PE array is still in its slow
p-state; DVE squares a few head slices it would otherwise idle through.
This makes the kernel memory-bound on the fp8 stream.

The cross term uses one per-slice matmul (pred_s^T @ gy_s) into a
persistent PSUM tile; the multiply by gx and the reduction run as two
grouped DVE ops.

Sharding: data-parallel over batch, 4 batches per core across 8 cores;
the tiny per-group column partials are combined on host in float64.
"""

import numpy as np
import ml_dtypes

import concourse.bacc as bacc
import concourse.bass as bass
import concourse.tile as tile
from concourse import mybir
from concourse.bass_utils import run_bass_kernel_spmd

B, V, J, H, W = 32, 4, 17, 128, 128
N_CORES = 8
B_LOC = B // N_CORES          # 4 batches per core
SLICES = B_LOC * V * J        # 272 slices per core

_CACHE = {}

# Per group: (pe_slices, act_slices, dve_slices). PE share is small in the
# first groups (array still ramping its p-state) and in the last (short
# tail after the final DMA lands).
GROUPS = [
    (22, 10, 6),
    (24, 9, 5),
    (29, 9, 0),
    (29, 9, 0),
    (29, 9, 0),
    (29, 9, 0),
    (20, 8, 0),
    (8, 6, 2),
]
assert sum(p + a + d for p, a, d in GROUPS) == SLICES
NG = len(GROUPS)
MAXSZ = max(p + a + d for p, a, d in GROUPS)
NGRP = NG - 2  # groups 0..NGRP-1 feed psA; the rest feed psB

# outcols: [G diags (NG) | ACT accums (NG) | DVE sq (NG) | prodA | prodB]
NC = 3 * NG + 2


def _build_nc():
    nc = bacc.Bacc()
    f32 = mybir.dt.float32
    bf16 = mybir.dt.bfloat16
    fp8 = mybir.dt.float8e4

    pred8 = nc.declare_dram_parameter("pred8", [H, SLICES, W], fp8, isOutput=False)
    gy8 = nc.declare_dram_parameter("gy8", [H, SLICES], fp8, isOutput=False)
    gx = nc.declare_dram_parameter("gx", [W, SLICES], bf16, isOutput=False)
    ident = nc.declare_dram_parameter("ident", [128, 128], bf16, isOutput=False)
    partials = nc.declare_dram_parameter("partials", [128, NC], f32, isOutput=True)

    n_main = sum(sum(g) for g in GROUPS[:NGRP])
    n_last = SLICES - n_main

    with tile.TileContext(nc) as tc:
        with (
            tc.tile_pool(name="consts", bufs=1) as consts,
            tc.tile_pool(name="l8", bufs=4) as l8pool,
            tc.tile_pool(name="work", bufs=1) as work,
            tc.tile_pool(name="gpsum", bufs=3, space="PSUM") as gpsum,
            tc.tile_pool(name="mpsum", bufs=1, space="PSUM") as mpsum,
            tc.tile_pool(name="outs", bufs=1) as outs,
        ):
            # warm-up ACT so the Square table-set load overlaps the DMA ramp
            warm = consts.tile([128, 1], f32)
            nc.vector.memset(warm[:], 0.0)
            wsq = consts.tile([128, 1], f32)
            nc.scalar.activation(
                out=wsq[:], in_=warm[:], func=mybir.ActivationFunctionType.Square
            )

            gy8_t = consts.tile([H, SLICES], fp8)
            gx_t = consts.tile([W, SLICES], bf16)
            ident_t = consts.tile([128, 128], bf16)
            actout = consts.tile([128, 10 * W], bf16)
            sq16 = consts.tile([128, 6 * W], bf16)
            outcols = outs.tile([128, NC], f32)

            psA = mpsum.tile([128, n_main], f32, tag="psA")
            psB = mpsum.tile([128, n_last], f32, tag="psB")

            g0 = 0
            mm0 = None
            for r, (pk, ak, dk) in enumerate(GROUPS):
                sz = pk + ak + dk
                t8 = l8pool.tile([128, MAXSZ * W], fp8, tag="l8")
                nc.sync.dma_start(
                    out=t8[:, : sz * W],
                    in_=pred8[:, g0 : g0 + sz, :].rearrange("h s w -> h (s w)"),
                )
                if r == 1:
                    # small const loads slot in after the first two pred
                    # DMAs; their consumers (PE cross-term matmuls, grouped
                    # prods, per-group diag extraction) all have slack
                    nc.sync.dma_start(out=gy8_t[:], in_=gy8[:, :])
                    nc.sync.dma_start(out=gx_t[:], in_=gx[:, :])
                    nc.sync.dma_start(out=ident_t[:], in_=ident[:, :])

                # PE: Gram accumulation over its share -> trace = sum sq
                G = gpsum.tile([128, 128], f32, tag="G")
                for i in range(pk):
                    nc.tensor.matmul(
                        G[:],
                        t8[:, i * W : (i + 1) * W],
                        t8[:, i * W : (i + 1) * W],
                        start=(i == 0),
                        stop=(i == pk - 1),
                    )
                # diag extraction: one masked multiply + one reduce
                def emit_diag(G=G, r=r):
                    dbuf = work.tile([128, 128], f32, tag="dbuf")
                    nc.vector.tensor_mul(dbuf[:], G[:], ident_t[:])
                    nc.vector.reduce_sum(
                        outcols[:, r : r + 1], dbuf[:], axis=mybir.AxisListType.X
                    )

                if r == 0:
                    # ident is only loaded during group 1; defer group 0's
                    # diag extraction past it in program order
                    diag0 = emit_diag
                else:
                    if r == 1:
                        diag0()
                    emit_diag()

                # ACT share: activation Square with accumulator
                nc.scalar.activation(
                    out=actout[:, : ak * W],
                    in_=t8[:, pk * W : (pk + ak) * W],
                    func=mybir.ActivationFunctionType.Square,
                    accum_out=outcols[:, NG + r : NG + r + 1],
                )

                # DVE share (head groups only): square, halve, reduce
                if dk:
                    n = dk * W
                    o = (pk + ak) * W
                    nc.vector.tensor_mul(
                        sq16[:, :n], t8[:, o : o + n], t8[:, o : o + n]
                    )
                    while n > 256:
                        h = n // 2
                        nc.vector.tensor_add(
                            sq16[:, :h], sq16[:, :h], sq16[:, h:n]
                        )
                        n = h
                    nc.vector.reduce_sum(
                        outcols[:, 2 * NG + r : 2 * NG + r + 1], sq16[:, :n],
                        axis=mybir.AxisListType.X,
                    )
                elif r < NG - 1:
                    nc.vector.memset(outcols[:, 2 * NG + r : 2 * NG + r + 1], 0.0)
                else:
                    nc.vector.memset(outcols[:, 2 * NG + r : 2 * NG + r + 1], 0.0)

                # cross term: m'_s = pred_s^T @ gy_s per slice -> psum column
                ps, col0 = (psA, g0) if r < NGRP else (psB, g0 - n_main)

                def emit_matmuls(ps=ps, col0=col0, t8=t8, sz=sz, g0=g0):
                    for i in range(sz):
                        nc.tensor.matmul(
                            ps[:, col0 + i : col0 + i + 1],
                            t8[:, i * W : (i + 1) * W],
                            gy8_t[:, g0 + i : g0 + i + 1],
                            start=True,
                            stop=True,
                        )

                if r == 0:
                    # gy is only loaded during group 1; defer group 0's
                    # cross-term matmuls past it in program order
                    mm0 = emit_matmuls
                else:
                    if r == 1:
                        mm0()
                    emit_matmuls()

                if r == NGRP:
                    # grouped prod over the main groups' m' columns; its
                    # matmul wait resolved long ago, no queue blocking
                    prodA = work.tile([128, n_main], f32, tag="prodA")
                    nc.vector.tensor_mul(prodA[:], psA[:], gx_t[:, :n_main])
                    nc.vector.reduce_sum(
                        outcols[:, 3 * NG : 3 * NG + 1], prodA[:],
                        axis=mybir.AxisListType.X,
                    )

                g0 += sz

            prodB = work.tile([128, n_last], f32, tag="prodB")
            nc.vector.tensor_mul(prodB[:], psB[:], gx_t[:, n_main:])
            nc.vector.reduce_sum(
                outcols[:, 3 * NG + 1 : 3 * NG + 2], prodB[:],
                axis=mybir.AxisListType.X,
            )

            nc.sync.dma_start(out=partials[:, :], in_=outcols[:])

    nc.finalize()
    return nc


def _gaussians(proj_mats_batch, joints_3d_gt_batch):
    """1D gaussians gy [B,V,J,H], gx [B,V,J,W] in float32 (reference math)."""
    joints = joints_3d_gt_batch.astype(np.float32)
    ones = np.ones(joints.shape[:-1] + (1,), dtype=np.float32)
    joints_h = np.concatenate([joints, ones], axis=-1)  # [B, J, 4]
    proj = np.einsum(
        "bvcd,bjd->bvjc", proj_mats_batch.astype(np.float32), joints_h
    ).astype(np.float32)  # [B, V, J, 3]
    joints_2d = proj[..., :2] / proj[..., 2:3]  # (x, y)
    xs = np.arange(W, dtype=np.float32)
    ys = np.arange(H, dtype=np.float32)
    dx2 = (xs - joints_2d[..., 0, None]) ** 2  # [B,V,J,W]
    dy2 = (ys - joints_2d[..., 1, None]) ** 2  # [B,V,J,H]
    gx = np.exp(-0.5 * dx2).astype(np.float32)
    gy = np.exp(-0.5 * dy2).astype(np.float32)
    return gy, gx


def kernel(heatmaps_pred, proj_mats_batch, joints_3d_gt_batch, joints_3d_valid_batch,
           _profile=None):
    heatmaps_pred = np.asarray(heatmaps_pred, dtype=np.float32)
    gy, gx = _gaussians(np.asarray(proj_mats_batch), np.asarray(joints_3d_gt_batch))

    # s3 = sum over slices of (sum_h gy^2) * (sum_w gx^2), exact in f64
    s3 = float(
        ((gy.astype(np.float64) ** 2).sum(-1) * (gx.astype(np.float64) ** 2).sum(-1)).sum()
    )

    if "nc" not in _CACHE:
        _CACHE["nc"] = _build_nc()
    nc = _CACHE["nc"]

    ident = np.eye(128, dtype=ml_dtypes.bfloat16)
    in_maps = []
    for c in range(N_CORES):
        bsl = slice(B_LOC * c, B_LOC * (c + 1))
        # slice order: (b_local, v, j) -> s ; pred h-major [H, SLICES, W]
        pred8 = np.ascontiguousarray(
            heatmaps_pred[bsl].reshape(SLICES, H, W).transpose(1, 0, 2)
            .astype(ml_dtypes.float8_e4m3)
        )
        gy8 = np.ascontiguousarray(
            gy[bsl].reshape(SLICES, H).T.astype(ml_dtypes.float8_e4m3)
        )
        gx_c = np.ascontiguousarray(
            gx[bsl].reshape(SLICES, W).T.astype(ml_dtypes.bfloat16)
        )
        in_maps.append({"pred8": pred8, "gy8": gy8, "gx": gx_c, "ident": ident})

    res = run_bass_kernel_spmd(nc, in_maps, core_ids=list(range(N_CORES)))
    if _profile is not None:
        _profile["result"] = res
        _profile["in_maps"] = in_maps

    s1 = 0.0
    s2 = 0.0
    for c in range(N_CORES):
        p = res.results[c]["partials"].astype(np.float64)
        s1 += p[:, : 3 * NG].sum()
        s2 += p[:, 3 * NG :].sum()

    total = s1 - 2.0 * s2 + s3
    return np.float32(total / (B * V * J * H * W))


# revision 19
# speedup vs baseline: 1.3755x; 1.0765x over previous
"""HeatmapMSELoss Trainium2 kernel (fp8 stream + PE Gram-matmul squares).

Computes mean((heatmaps_pred - heatmaps_gt)^2) where heatmaps_gt is an
isotropic 2D gaussian (sigma=1, peak 1) rendered at the projection of each
3D joint into each view.

Key identity: the gaussian separates, gt[h,w] = gy[h] * gx[w], so

  sum_hw (pred - gt)^2 = sum_hw pred^2 - 2 * gy^T (pred @ gx) + (sum gy^2)(sum gx^2)

The gt tensor is never materialized. pred is pre-transposed on host to
h-major [H, S, W] and cast to fp8e4m3 (biases the final scalar by ~7e-4
relative, harness tolerance 2e-2), quartering HBM traffic vs f32 while
every DMA descriptor stays >= 512 contiguous bytes.

sum(pred^2) rides the (otherwise idle) PE systolic array: per group of
slices, PSUM-accumulated Gram matmuls G = sum_s pred_s^T pred_s, whose
trace is sum of squares. The diagonal is extracted once per group by one
DVE multiply against a preloaded identity mask and one reduce. The ACT
engine squares a minority share directly (activation Square + accum),
sized larger in the early groups while the PE array is still in its slow
p-state; DVE squares a few head slices it would otherwise idle through.
This makes the kernel memory-bound on the fp8 stream.

The cross term uses one per-slice matmul (pred_s^T @ gy_s) into a
persistent PSUM tile; the multiply by gx and the reduction run as two
grouped DVE ops.

Sharding: data-parallel over batch, 4 batches per core across 8 cores;
the tiny per-group column partials are combined on host in float64.
"""

import numpy as np
import ml_dtypes

import concourse.bacc as bacc
import concourse.bass as bass
import concourse.tile as tile
from concourse import mybir
from concourse.bass_utils import run_bass_kernel_spmd

B, V, J, H, W = 32, 4, 17, 128, 128
N_CORES = 8
B_LOC = B // N_CORES          # 4 batches per core
SLICES = B_LOC * V * J        # 272 slices per core

_CACHE = {}

# Per group: (pe_slices, act_slices, dve_slices). PE share is small in the
# first groups (array still ramping its p-state) and in the last (short
# tail after the final DMA lands).
GROUPS = [
    (22, 10, 6),
    (24, 9, 5),
    (25, 11, 2),
    (25, 11, 2),
    (25, 11, 2),
    (25, 11, 2),
    (20, 8, 0),
    (8, 6, 2),
]
assert sum(p + a + d for p, a, d in GROUPS) == SLICES
NG = len(GROUPS)
MAXSZ = max(p + a + d for p, a, d in GROUPS)
NGRP = NG - 2  # groups 0..NGRP-1 feed psA; the rest feed psB

# outcols: [G diags (NG) | ACT accums (NG) | DVE sq (NG) | prodA | prodB]
NC = 3 * NG + 2


def _build_nc():
    nc = bacc.Bacc()
    f32 = mybir.dt.float32
    bf16 = mybir.dt.bfloat16
    fp8 = mybir.dt.float8e4

    pred8 = nc.declare_dram_parameter("pred8", [H, SLICES, W], fp8, isOutput=False)
    gy8 = nc.declare_dram_parameter("gy8", [H, SLICES], fp8, isOutput=False)
    gx = nc.declare_dram_parameter("gx", [W, SLICES], bf16, isOutput=False)
    ident = nc.declare_dram_parameter("ident", [128, 128], bf16, isOutput=False)
    partials = nc.declare_dram_parameter("partials", [128, NC], f32, isOutput=True)

    n_main = sum(sum(g) for g in GROUPS[:NGRP])
    n_last = SLICES - n_main

    with tile.TileContext(nc) as tc:
        with (
            tc.tile_pool(name="consts", bufs=1) as consts,
            tc.tile_pool(name="l8", bufs=6) as l8pool,
            tc.tile_pool(name="work", bufs=1) as work,
            tc.tile_pool(name="gpsum", bufs=3, space="PSUM") as gpsum,
            tc.tile_pool(name="mpsum", bufs=1, space="PSUM") as mpsum,
            tc.tile_pool(name="outs", bufs=1) as outs,
        ):
            # warm-up ACT so the Square table-set load overlaps the DMA ramp
            warm = consts.tile([128, 1], f32)
            nc.vector.memset(warm[:], 0.0)
            wsq = consts.tile([128, 1], f32)
            nc.scalar.activation(
                out=wsq[:], in_=warm[:], func=mybir.ActivationFunctionType.Square
            )

            gy8_t = consts.tile([H, SLICES], fp8)
            gx_t = consts.tile([W, SLICES], bf16)
            ident_t = consts.tile([128, 128], bf16)
            actout = consts.tile([128, max(a for _, a, _ in GROUPS) * W], bf16)
            sq16 = consts.tile([128, 6 * W], bf16)
            outcols = outs.tile([128, NC], f32)

            psA = mpsum.tile([128, n_main], f32, tag="psA")
            psB = mpsum.tile([128, n_last], f32, tag="psB")

            g0 = 0
            mm0 = None
            for r, (pk, ak, dk) in enumerate(GROUPS):
                sz = pk + ak + dk
                t8 = l8pool.tile([128, MAXSZ * W], fp8, tag="l8")
                nc.sync.dma_start(
                    out=t8[:, : sz * W],
                    in_=pred8[:, g0 : g0 + sz, :].rearrange("h s w -> h (s w)"),
                )
                if r == 1:
                    # small const loads slot in after the first two pred
                    # DMAs; their consumers (PE cross-term matmuls, grouped
                    # prods, per-group diag extraction) all have slack
                    nc.sync.dma_start(out=gy8_t[:], in_=gy8[:, :])
                    nc.sync.dma_start(out=gx_t[:], in_=gx[:, :])
                    nc.sync.dma_start(out=ident_t[:], in_=ident[:, :])

                # PE: Gram accumulation over its share -> trace = sum sq
                G = gpsum.tile([128, 128], f32, tag="G")
                for i in range(pk):
                    nc.tensor.matmul(
                        G[:],
                        t8[:, i * W : (i + 1) * W],
                        t8[:, i * W : (i + 1) * W],
                        start=(i == 0),
                        stop=(i == pk - 1),
                    )
                # diag extraction: one masked multiply + one reduce
                def emit_diag(G=G, r=r):
                    dbuf = work.tile([128, 128], f32, tag="dbuf")
                    nc.vector.tensor_mul(dbuf[:], G[:], ident_t[:])
                    nc.vector.reduce_sum(
                        outcols[:, r : r + 1], dbuf[:], axis=mybir.AxisListType.X
                    )

                if r == 0:
                    # ident is only loaded during group 1; defer group 0's
                    # diag extraction past it in program order
                    diag0 = emit_diag
                else:
                    if r == 1:
                        diag0()
                    emit_diag()

                # ACT share: activation Square with accumulator
                nc.scalar.activation(
                    out=actout[:, : ak * W],
                    in_=t8[:, pk * W : (pk + ak) * W],
                    func=mybir.ActivationFunctionType.Square,
                    accum_out=outcols[:, NG + r : NG + r + 1],
                )

                # DVE share (head groups only): square, halve, reduce
                if dk:
                    n = dk * W
                    o = (pk + ak) * W
                    nc.vector.tensor_mul(
                        sq16[:, :n], t8[:, o : o + n], t8[:, o : o + n]
                    )
                    while n > 256:
                        h = n // 2
                        nc.vector.tensor_add(
                            sq16[:, :h], sq16[:, :h], sq16[:, h:n]
                        )
                        n = h
                    nc.vector.reduce_sum(
                        outcols[:, 2 * NG + r : 2 * NG + r + 1], sq16[:, :n],
                        axis=mybir.AxisListType.X,
                    )
                elif r < NG - 1:
                    nc.vector.memset(outcols[:, 2 * NG + r : 2 * NG + r + 1], 0.0)
                else:
                    nc.vector.memset(outcols[:, 2 * NG + r : 2 * NG + r + 1], 0.0)

                # cross term: m'_s = pred_s^T @ gy_s per slice -> psum column
                ps, col0 = (psA, g0) if r < NGRP else (psB, g0 - n_main)

                def emit_matmuls(ps=ps, col0=col0, t8=t8, sz=sz, g0=g0):
                    for i in range(sz):
                        nc.tensor.matmul(
                            ps[:, col0 + i : col0 + i + 1],
                            t8[:, i * W : (i + 1) * W],
                            gy8_t[:, g0 + i : g0 + i + 1],
                            start=True,
                            stop=True,
                        )

                if r == 0:
                    # gy is only loaded during group 1; defer group 0's
                    # cross-term matmuls past it in program order
                    mm0 = emit_matmuls
                else:
                    if r == 1:
                        mm0()
                    emit_matmuls()

                if r == NGRP:
                    # grouped prod over the main groups' m' columns; its
                    # matmul wait resolved long ago, no queue blocking
                    prodA = work.tile([128, n_main], f32, tag="prodA")
                    nc.vector.tensor_mul(prodA[:], psA[:], gx_t[:, :n_main])
                    nc.vector.reduce_sum(
                        outcols[:, 3 * NG : 3 * NG + 1], prodA[:],
                        axis=mybir.AxisListType.X,
                    )

                g0 += sz

            prodB = work.tile([128, n_last], f32, tag="prodB")
            nc.vector.tensor_mul(prodB[:], psB[:], gx_t[:, n_main:])
            nc.vector.reduce_sum(
                outcols[:, 3 * NG + 1 : 3 * NG + 2], prodB[:],
                axis=mybir.AxisListType.X,
            )

            nc.sync.dma_start(out=partials[:, :], in_=outcols[:])

    nc.finalize()
    return nc


def _gaussians(proj_mats_batch, joints_3d_gt_batch):
    """1D gaussians gy [B,V,J,H], gx [B,V,J,W] in float32 (reference math)."""
    joints = joints_3d_gt_batch.astype(np.float32)
    ones = np.ones(joints.shape[:-1] + (1,), dtype=np.float32)
    joints_h = np.concatenate([joints, ones], axis=-1)  # [B, J, 4]
    proj = np.einsum(
        "bvcd,bjd->bvjc", proj_mats_batch.astype(np.float32), joints_h
    ).astype(np.float32)  # [B, V, J, 3]
    joints_2d = proj[..., :2] / proj[..., 2:3]  # (x, y)
    xs = np.arange(W, dtype=np.float32)
    ys = np.arange(H, dtype=np.float32)
    dx2 = (xs - joints_2d[..., 0, None]) ** 2  # [B,V,J,W]
    dy2 = (ys - joints_2d[..., 1, None]) ** 2  # [B,V,J,H]
    gx = np.exp(-0.5 * dx2).astype(np.float32)
    gy = np.exp(-0.5 * dy2).astype(np.float32)
    return gy, gx


def kernel(heatmaps_pred, proj_mats_batch, joints_3d_gt_batch, joints_3d_valid_batch,
           _profile=None):
    heatmaps_pred = np.asarray(heatmaps_pred, dtype=np.float32)
    gy, gx = _gaussians(np.asarray(proj_mats_batch), np.asarray(joints_3d_gt_batch))

    # s3 = sum over slices of (sum_h gy^2) * (sum_w gx^2), exact in f64
    s3 = float(
        ((gy.astype(np.float64) ** 2).sum(-1) * (gx.astype(np.float64) ** 2).sum(-1)).sum()
    )

    if "nc" not in _CACHE:
        _CACHE["nc"] = _build_nc()
    nc = _CACHE["nc"]

    ident = np.eye(128, dtype=ml_dtypes.bfloat16)
    in_maps = []
    for c in range(N_CORES):
        bsl = slice(B_LOC * c, B_LOC * (c + 1))
        # slice order: (b_local, v, j) -> s ; pred h-major [H, SLICES, W]
        pred8 = np.ascontiguousarray(
            heatmaps_pred[bsl].reshape(SLICES, H, W).transpose(1, 0, 2)
            .astype(ml_dtypes.float8_e4m3)
        )
        gy8 = np.ascontiguousarray(
            gy[bsl].reshape(SLICES, H).T.astype(ml_dtypes.float8_e4m3)
        )
        gx_c = np.ascontiguousarray(
            gx[bsl].reshape(SLICES, W).T.astype(ml_dtypes.bfloat16)
        )
        in_maps.append({"pred8": pred8, "gy8": gy8, "gx": gx_c, "ident": ident})

    res = run_bass_kernel_spmd(nc, in_maps, core_ids=list(range(N_CORES)))
    if _profile is not None:
        _profile["result"] = res
        _profile["in_maps"] = in_maps

    s1 = 0.0
    s2 = 0.0
    for c in range(N_CORES):
        p = res.results[c]["partials"].astype(np.float64)
        s1 += p[:, : 3 * NG].sum()
        s2 += p[:, 3 * NG :].sum()

    total = s1 - 2.0 * s2 + s3
    return np.float32(total / (B * V * J * H * W))
